# revision 1
# baseline (speedup 1.0000x reference)
"""CVQNN classifier kernel for 8 Trainium2 NeuronCores.

Math: the whole quantum circuit collapses to a batch-independent affine map
(S, d) on 128-dim phase space.  Per batch row the heavy work is
    m = x @ W2 + d20          (W2 = S[rows, :64].T, shape (64, 20))
    out_k = log1p(m_x[k]^2 + m_p[k]^2 + covc_k)
i.e. a (B,64) @ (64,20) matmul + elementwise tail -> (B,10).  Memory bound.
(The reference's relu is a provable no-op: covc >= 0 for symplectic S.)

The rel-err budget (2e-2) admits pure bf16 x and W (measured 2.9e-3), so
unlike the previous hi/lo-split version this one moves HALF the input
bytes and also returns bf16 outputs (host upcasts).

Device layout (per core, R = 125952 rows, 2 rows per xstack column):
  - xstack (128, 62976) bf16: partitions 0..63 = features of row-group A,
    64..127 = features of row-group B (consecutive 128*w-row chunks of
    each super-block).  Block-diagonal weights wcat (128, 40) =
    [[Wh, 0], [0, Wh]] make one 128x128-stationary matmul produce
    40 psum cols = 10 classes x {x,p} x {A,B} for 256 rows.
  - per super-block (48 j-blocks = 12288 rows): 1 input DMA [128, 6144]
    (12 KB per-partition lines), 4 bank-preload matmuls (stationary =
    ones/128, moving = d-pattern, N=480) fold the +d into PSUM, then 48
    accumulating matmuls (start=False).  No DVE bias-add needed.
  - tail: ACT square (psum -> sbuf), DVE pair-add + cov-add, ACT ln(1+.)
    straight to bf16.  ln(s-1) is emitted after square(s) so the in-order
    ACT queue never stalls on the DVE chain.
  - all DMA on the two HWDGE rings (input on sync/SP, output on
    scalar/ACT) - no SWDGE descriptor-ring traffic at all.
"""

import ml_dtypes
import numpy as np

import concourse.bacc as bacc
import concourse.mybir as mybir
import concourse.tile as tile
from concourse.bass_utils import run_bass_kernel_spmd

N = 64          # wires
OUT = 10        # measured wires / classes
NCORES = 8
JBLK = 48                  # j-blocks per full super-block (4 psum banks)
# j-block = 256 rows (2 row-groups x 128).  Small first sb so the tail
# pipeline starts almost immediately (no standing lag); small last sb for
# a short drain.  492 j-blocks, 125952 rows/core, 0.76% padding.
WIDTHS = [12] + [JBLK] * 9 + [36, 12]
JTOT = sum(WIDTHS)         # 492
R = 256 * JTOT             # per-core rows = 125952
CC = 128 * JTOT            # per-core xstack cols = 62976
B_PAD = R * NCORES         # 1007616
F32 = mybir.dt.float32
BF16 = mybir.dt.bfloat16
NPBF16 = ml_dtypes.bfloat16


# ---------------------------------------------------------------- host math
def _bs_pass(n, start, int_params):
    i = np.arange(start, n - 1, 2)
    j = i + 1
    theta = int_params[3 * i]
    phi = int_params[3 * i + 1]
    ct, st = np.cos(theta), np.sin(theta)
    cp, sp = np.cos(phi), np.sin(phi)
    S = np.eye(2 * n)
    S[i, i] = ct
    S[i, j] = -cp * st
    S[i, n + j] = -sp * st
    S[j, i] = cp * st
    S[j, j] = ct
    S[j, n + i] = -sp * st
    S[n + i, j] = sp * st
    S[n + i, n + i] = ct
    S[n + i, n + j] = -cp * st
    S[n + j, i] = sp * st
    S[n + j, n + i] = cp * st
    S[n + j, n + j] = ct
    return S


def _layer_symplectic(n, int1, squeezes, int2):
    M = _bs_pass(n, 0, int1)
    M = _bs_pass(n, 1, int1) @ M
    c = np.concatenate([np.cos(int1[2::3]), np.ones(1)])
    s = np.concatenate([np.sin(int1[2::3]), np.zeros(1)])
    Rm = np.block([[np.diag(c), np.diag(-s)], [np.diag(s), np.diag(c)]])
    Sq = np.diag(np.concatenate([np.exp(-squeezes), np.exp(squeezes)]))
    M = Sq @ (Rm @ M)
    M = _bs_pass(n, 0, int2) @ M
    M = _bs_pass(n, 1, int2) @ M
    return M


def _affine_map(layers):
    n = N
    S = np.eye(2 * n)
    d = np.zeros(2 * n)
    for int1, sq, int2, disp in layers:
        M = _layer_symplectic(n, int1, sq, int2)
        S = M @ S
        d = M @ d
        d[:n] += 2.0 * disp
    return S, d


def _device_constants(layers):
    S, d = _affine_map(layers)
    w = np.arange(OUT)
    rows = np.concatenate([w, N + w])
    cov = S @ S.T
    cov_term = cov[w, w] + cov[N + w, N + w]            # (10,)
    W2 = S[rows, :N].T                                  # (64, 20)
    d20 = d[rows] / 2.0                                 # (20,)
    covc = np.maximum(cov_term / 4.0 - 0.5, 0.0)

    # ln1p(mx^2+mp^2+covc) = ln1p((mx^2+mp^2)*a) + ln1p(covc), a=1/(1+covc)
    # fold sqrt(a) into W and d so the device never adds covc; the host
    # adds beta = ln1p(covc) after decode.  Kills one DVE op per sb and,
    # with it, the ACT<->DVE round-trip that was pacing the pipeline.
    r20 = np.tile(np.sqrt(1.0 / (1.0 + covc)), 2)       # (20,)
    beta = np.log1p(covc).astype(np.float32)            # (10,)
    Wp = (W2 * r20).astype(np.float32)
    dp = (d20 * r20).astype(np.float32)

    Wh = Wp.astype(NPBF16)
    wcat = np.zeros((128, 40), NPBF16)                  # [[Wh, 0], [0, Wh]]
    wcat[0:64, 0:20] = Wh
    wcat[64:128, 20:40] = Wh

    ones = np.ones((128, 128), NPBF16)
    # bank preload pattern: 12 slots x [d | d]; moving operand is d/128 so
    # the 128-partition ones-contraction reconstitutes d exactly
    dpat = np.ascontiguousarray(np.broadcast_to(
        np.tile(dp, 24) / 128.0, (128, 480))).astype(NPBF16)
    return wcat, ones, dpat, beta


# ---------------------------------------------------------------- bass build
def build_nc(widths=None):
    widths = widths or WIDTHS
    jtot = sum(widths)
    nc = bacc.Bacc("TRN2", target_bir_lowering=False)
    xs = nc.dram_tensor("xs", (128, 128 * jtot), BF16, kind="ExternalInput")
    wst = nc.dram_tensor("wcat", (128, 40), BF16, kind="ExternalInput")
    onest = nc.dram_tensor("ones", (128, 128), BF16, kind="ExternalInput")
    dpatt = nc.dram_tensor("dpat", (128, 480), BF16, kind="ExternalInput")
    out = nc.dram_tensor("out", (128, 20 * jtot), BF16, kind="ExternalOutput")

    Square = mybir.ActivationFunctionType.Square
    Ln = mybir.ActivationFunctionType.Ln

    with tile.TileContext(nc) as tc:
        with (
            tc.tile_pool(name="const", bufs=1) as cpool,
            tc.tile_pool(name="xin", bufs=4) as xpool,
            tc.tile_pool(name="mid", bufs=3) as mpool,
            tc.tile_pool(name="ob", bufs=8) as opool,
            tc.tile_pool(name="ps", bufs=2, space="PSUM") as pspool,
        ):
            # consts gate the first matmuls: load on the input (sync) ring
            w_t = cpool.tile([128, 40], BF16)
            ones_t = cpool.tile([128, 128], BF16)
            dpat_t = cpool.tile([128, 480], BF16)

            def load_consts():
                nc.sync.dma_start(w_t[:], wst[:])
                nc.sync.dma_start(ones_t[:], onest[:])
                nc.sync.dma_start(dpat_t[:], dpatt[:])

            pending = []

            def flush_pending(keep=0):
                # ln(s-2) runs here, two super-blocks behind: its input (v)
                # is long since ready, so the in-order ACT queue never waits
                # on the DVE chain.  Output DMA issues from gpsimd (SWDGE)
                # to keep the ACT queue free for square/ln.
                while len(pending) > keep:
                    v, oc, ob = pending.pop(0)
                    o = opool.tile([128, oc], BF16, tag="o")
                    nc.scalar.activation(o[:], v[:], Ln, bias=1.0)
                    nc.gpsimd.dma_start(out[:, ob:ob + oc], o[:])

            def emit_sb(col_base, jblk, in_chunks):
                wc, oc, nbank = 40 * jblk, 20 * jblk, jblk // 12
                w = 128 * jblk
                tin = xpool.tile([128, w], BF16, tag="tin")
                q = w // in_chunks
                for c4 in range(in_chunks):
                    # alternate the two HWDGE rings (sync=SP, scalar=ACT):
                    # the halves transfer concurrently and matmuls start
                    # while the later half is still in flight
                    eng = nc.sync if c4 % 2 == 0 else nc.scalar
                    eng.dma_start(
                        tin[:, c4 * q:(c4 + 1) * q],
                        xs[:, col_base + c4 * q:col_base + (c4 + 1) * q])
                    if col_base == 0 and c4 == 0:
                        # consts ride the sync ring right behind the first
                        # input chunk: stream starts earlier, matmuls still
                        # gated only ~1us later
                        load_consts()

                # psum: 12 j-blocks in the first 480 cols of each bank;
                # bank preload folds +d into the accumulation
                ps = pspool.tile([128, nbank, 512], F32, tag="ps")
                for t in range(nbank):
                    nc.tensor.matmul(ps[:, t, 0:480], ones_t[:], dpat_t[:],
                                     start=True, stop=False)
                for j in range(jblk):
                    nc.tensor.matmul(
                        ps[:, j // 12, 40 * (j % 12):40 * (j % 12) + 40],
                        tin[:, 128 * j:128 * j + 128], w_t[:],
                        start=False, stop=True,
                    )

                # software-pipelined: prev chunks' ln+store go BEFORE our
                # squares on the ACT queue, right after their true producer
                # (the DVE pair-add) was emitted — Tile's cross-engine waits
                # are engine-barriers, so emission adjacency is what matters
                flush_pending(keep=0)

                # tail squares/pairs in <=2-bank chunks (the ACT-DVE-ACT
                # recurrence is the pipeline's pace setter; smaller ops
                # start sooner and overlap), but one ln + one store per sb
                v = mpool.tile([128, oc], F32, tag="v")
                t0 = 0
                while t0 < nbank:
                    tn = min(2, nbank - t0)
                    cwc = tn * 480
                    sq = mpool.tile([128, cwc], F32, tag=f"sq{t0}")
                    sqv = sq[:].rearrange("p (t c) -> p t c", t=tn)
                    nc.scalar.activation(sqv, ps[:, t0:t0 + tn, 0:480], Square)

                    sq2 = sq[:].rearrange("p (g xp k) -> p g xp k", xp=2, k=10)
                    sv = v[:, t0 * 240:t0 * 240 + tn * 240].rearrange(
                        "p (g k) -> p g k", k=10)
                    nc.vector.tensor_add(sv, sq2[:, :, 0, :], sq2[:, :, 1, :])
                    t0 += tn
                pending.append((v, oc, (col_base // 128) * 20))

            # input DMAs split so matmuls start before the whole tile lands
            col = 0
            for i, wdt in enumerate(widths):
                emit_sb(col, wdt, 4 if i == 0 else 2)
                col += 128 * wdt
            flush_pending()
    nc.compile()
    return nc


# ---------------------------------------------------------------- host glue
def _make_in_maps(x_batch, wcat, ones, dpat):
    B = x_batch.shape[0]
    xpad = np.zeros((B_PAD, N), np.float32)
    xpad[:B] = x_batch
    xh = xpad.astype(NPBF16)
    in_maps = []
    for c in range(NCORES):
        xc = xh[c * R:(c + 1) * R]
        xstk = np.empty((128, CC), NPBF16)
        # per sb: rows (grp, t, f) -> xstk[grp*64+f, c0+t]
        r0 = c0 = 0
        for w in WIDTHS:
            half = 128 * w
            xt = xc[r0:r0 + 2 * half].reshape(2, half, N)
            xstk[:, c0:c0 + half] = xt.transpose(0, 2, 1).reshape(128, half)
            r0 += 2 * half
            c0 += half
        in_maps.append({"xs": xstk, "wcat": wcat, "ones": ones,
                        "dpat": dpat})
    return in_maps


def _decode_out(results, B, beta):
    full = np.empty((B_PAD, OUT), np.float32)
    for c in range(NCORES):
        O = results[c]["out"].astype(np.float32)
        r0 = o0 = 0
        for w in WIDTHS:
            Ow = O[:, o0:o0 + 20 * w].reshape(128, w, 2, OUT)
            full[c * R + r0:c * R + r0 + 256 * w] = (
                Ow.transpose(2, 1, 0, 3).reshape(256 * w, OUT))
            r0 += 256 * w
            o0 += 20 * w
    full += beta            # ln1p(covc), factored off the device
    return full[:B]


_NC_CACHE = {}


def kernel(x_batch, int1_0, squeezes_0, int2_0, disp_0,
           int1_1, squeezes_1, int2_1, disp_1, _trace=False):
    layers = [
        (np.asarray(int1_0, np.float64), np.asarray(squeezes_0, np.float64),
         np.asarray(int2_0, np.float64), np.asarray(disp_0, np.float64)),
        (np.asarray(int1_1, np.float64), np.asarray(squeezes_1, np.float64),
         np.asarray(int2_1, np.float64), np.asarray(disp_1, np.float64)),
    ]
    wcat, ones, dpat, beta = _device_constants(layers)
    in_maps = _make_in_maps(np.asarray(x_batch, np.float32),
                            wcat, ones, dpat)

    if "nc" not in _NC_CACHE:
        _NC_CACHE["nc"] = build_nc()
    nc = _NC_CACHE["nc"]

    res = run_bass_kernel_spmd(
        nc, in_maps, core_ids=list(range(NCORES)), trace=_trace
    )
    out = _decode_out(res.results, x_batch.shape[0], beta)
    if _trace:
        return out, res
    return out



# revision 5
# speedup vs baseline: 1.6349x; 1.6349x over previous
"""CVQNN classifier kernel for 8 Trainium2 NeuronCores — v3 (fp8 input).

Math: the whole quantum circuit collapses to a batch-independent affine map
(S, d) on 128-dim phase space.  Per batch row the heavy work is
    m = x @ W2 + d20          (W2 = S[rows, :64].T, shape (64, 20))
    out_k = log1p(m_x[k]^2 + m_p[k]^2 + covc_k)
i.e. a (B,64) @ (64,20) matmul + elementwise tail -> (B,10).  Memory bound.

v3 vs the bf16 baseline (88.3us):
  - x ships as float8 e3m4 (max 15.5, ~1% RMS quant err; e4m3 would blow the
    2e-2 budget, e3m4 measures 1.45e-2 end-to-end) -> input bytes halve.
    Weights stay bf16 (mixed-dtype matmul is legal on TRN2).
  - the whole per-core input (61.5 KiB/partition) stays resident in SBUF:
    every super-block gets its own tile, no reuse, so input DMA is never
    backpressured by compute.  All input DMA rides the sync/SP HWDGE ring
    only; one ring saturates HBM fine and the ACT queue stays clean for
    compute (the baseline's trace showed input DMA idling 40% of the time
    and every engine <55% busy - it was dependency-bound, not BW-bound).
  - tail is split: ACT squares the x-half (psum cols are (xp,g,k) so each
    half is one 4-dim AP slice), DVE squares the p-half (tensor mult) and
    does the pair-add in bf16 (2x mode).  ACT also does ln1p.  This keeps
    ACT ~2.1us/sb and DVE ~1.9us/sb, both under the 2.9us/sb DMA floor.
  - d-preload matmuls (ones @ dpat -> psum, j-matmuls accumulate on top)
    kept from the baseline: the "fold d into a shift of x" trick is
    impossible (S[rows,:64] is exactly rank-deficient, residual 3e-2).
    With the tight pipeline the PE stays HAM-warm so they cost ~0.85us/sb.

Device layout (per core, R = 125952 rows, 2 rows per xstack column):
  - xstack (128, 62976) fp8e3: partitions 0..63 = features of row-group A,
    64..127 = row-group B (consecutive 128*w-row chunks per super-block).
    Block-diagonal wcat (128, 40), column order (xp*20 + g*10 + k), makes
    one 128x128-stationary matmul produce 40 psum cols for 256 rows.
  - per super-block (48 j-blocks = 12288 rows): 4 bank-preload matmuls
    (ones/128 x d-pattern, N=480) fold +d into PSUM, then 48 accumulating
    matmuls (start=False).
  - tail per 2-bank chunk: ACT Square (x-half) -> bf16 sbuf, DVE mult
    (p-half) -> bf16 sbuf, DVE add -> v (bf16, 2x mode).  ln(1+v) for
    super-block N is emitted during super-block N+1 so the in-order ACT
    queue never stalls on the DVE chain.  Output bf16 via gpsimd (SWDGE).
"""

import ml_dtypes
import numpy as np

import concourse.bacc as bacc
import concourse.mybir as mybir
import concourse.tile as tile
from concourse.bass_utils import run_bass_kernel_spmd

N = 64          # wires
OUT = 10        # measured wires / classes
NCORES = 8
JBLK = 48                  # j-blocks per full super-block (4 psum banks)
WIDTHS = [12] + [JBLK] * 9 + [36, 12]
JTOT = sum(WIDTHS)         # 492
R = 256 * JTOT             # per-core rows = 125952
CC = 128 * JTOT            # per-core xstack cols = 62976
B_PAD = R * NCORES         # 1007616
F32 = mybir.dt.float32
BF16 = mybir.dt.bfloat16
FP8 = mybir.dt.float8e3
NPBF16 = ml_dtypes.bfloat16
NPFP8 = ml_dtypes.float8_e3m4


# ---------------------------------------------------------------- host math
def _bs_pass(n, start, int_params):
    i = np.arange(start, n - 1, 2)
    j = i + 1
    theta = int_params[3 * i]
    phi = int_params[3 * i + 1]
    ct, st = np.cos(theta), np.sin(theta)
    cp, sp = np.cos(phi), np.sin(phi)
    S = np.eye(2 * n)
    S[i, i] = ct
    S[i, j] = -cp * st
    S[i, n + j] = -sp * st
    S[j, i] = cp * st
    S[j, j] = ct
    S[j, n + i] = -sp * st
    S[n + i, j] = sp * st
    S[n + i, n + i] = ct
    S[n + i, n + j] = -cp * st
    S[n + j, i] = sp * st
    S[n + j, n + i] = cp * st
    S[n + j, n + j] = ct
    return S


def _layer_symplectic(n, int1, squeezes, int2):
    M = _bs_pass(n, 0, int1)
    M = _bs_pass(n, 1, int1) @ M
    c = np.concatenate([np.cos(int1[2::3]), np.ones(1)])
    s = np.concatenate([np.sin(int1[2::3]), np.zeros(1)])
    Rm = np.block([[np.diag(c), np.diag(-s)], [np.diag(s), np.diag(c)]])
    Sq = np.diag(np.concatenate([np.exp(-squeezes), np.exp(squeezes)]))
    M = Sq @ (Rm @ M)
    M = _bs_pass(n, 0, int2) @ M
    M = _bs_pass(n, 1, int2) @ M
    return M


def _affine_map(layers):
    n = N
    S = np.eye(2 * n)
    d = np.zeros(2 * n)
    for int1, sq, int2, disp in layers:
        M = _layer_symplectic(n, int1, sq, int2)
        S = M @ S
        d = M @ d
        d[:n] += 2.0 * disp
    return S, d


def _device_constants(layers):
    S, d = _affine_map(layers)
    w = np.arange(OUT)
    rows = np.concatenate([w, N + w])
    cov = S @ S.T
    cov_term = cov[w, w] + cov[N + w, N + w]            # (10,)
    W2 = S[rows, :N].T                                  # (64, 20): (xp,k)
    d20 = d[rows] / 2.0                                 # (20,)
    covc = np.maximum(cov_term / 4.0 - 0.5, 0.0)

    # ln1p(mx^2+mp^2+covc) = ln1p((mx^2+mp^2)*a) + ln1p(covc), a=1/(1+covc)
    r20 = np.tile(np.sqrt(1.0 / (1.0 + covc)), 2)       # (20,)
    beta = np.log1p(covc).astype(np.float32)            # (10,)
    Wp = (W2 * r20).astype(np.float32)                  # (64, 20)
    dp = (d20 * r20).astype(np.float32)                 # (20,)

    # wcat column order (xp, g, k): col = xp*20 + g*10 + k
    Wh = Wp.astype(NPBF16)
    wcat = np.zeros((128, 40), NPBF16)
    for xp in range(2):
        wcat[0:64, xp * 20 + 0:xp * 20 + 10] = Wh[:, xp * 10:xp * 10 + 10]
        wcat[64:128, xp * 20 + 10:xp * 20 + 20] = Wh[:, xp * 10:xp * 10 + 10]

    # d pattern in matching order: d40[xp*20 + g*10 + k] = dp[xp*10+k]
    d40 = np.empty(40, np.float64)
    for xp in range(2):
        for g in range(2):
            d40[xp * 20 + g * 10:xp * 20 + g * 10 + 10] = (
                dp[xp * 10:xp * 10 + 10])
    ones = np.ones((128, 128), NPBF16)
    # bank preload pattern: 12 slots x d40; moving operand is d/128 so the
    # 128-partition ones-contraction reconstitutes d exactly
    dpat = np.ascontiguousarray(np.broadcast_to(
        np.tile(d40, 12) / 128.0, (128, 480))).astype(NPBF16)
    return wcat, ones, dpat, beta


# ---------------------------------------------------------------- bass build
def build_nc(widths=None):
    widths = widths or WIDTHS
    jtot = sum(widths)
    nsb = len(widths)
    nc = bacc.Bacc("TRN2", target_bir_lowering=False)
    xs = nc.dram_tensor("xs", (128, 128 * jtot), FP8, kind="ExternalInput")
    wst = nc.dram_tensor("wcat", (128, 40), BF16, kind="ExternalInput")
    onest = nc.dram_tensor("ones", (128, 128), BF16, kind="ExternalInput")
    dpatt = nc.dram_tensor("dpat", (128, 480), BF16, kind="ExternalInput")
    out = nc.dram_tensor("out", (128, 20 * jtot), BF16, kind="ExternalOutput")

    Square = mybir.ActivationFunctionType.Square
    Ln = mybir.ActivationFunctionType.Ln

    with tile.TileContext(nc) as tc:
        with (
            tc.tile_pool(name="const", bufs=1) as cpool,
            tc.tile_pool(name="xin", bufs=1) as xpool,
            tc.tile_pool(name="mid", bufs=3) as mpool,
            tc.tile_pool(name="ob", bufs=8) as opool,
            tc.tile_pool(name="ps", bufs=2, space="PSUM") as pspool,
        ):
            w_t = cpool.tile([128, 40], BF16)
            ones_t = cpool.tile([128, 128], BF16)
            dpat_t = cpool.tile([128, 480], BF16)

            # one resident tile per super-block; never reused, so the DMA
            # stream is gated only by the ring itself
            xtiles = []
            col = 0
            for i, wdt in enumerate(widths):
                xtiles.append(xpool.tile([128, 128 * wdt], FP8,
                                         tag=f"x{i}", name=f"x{i}"))
                col += 128 * wdt

            def issue_input(i, col_base, wdt):
                w = 128 * wdt
                nc.sync.dma_start(xtiles[i][:], xs[:, col_base:col_base + w])
                if i == 0:
                    # consts ride the sync ring right behind the first
                    # (small) input tile: matmuls gate only ~1us later
                    nc.sync.dma_start(w_t[:], wst[:])
                    nc.sync.dma_start(ones_t[:], onest[:])
                    nc.sync.dma_start(dpat_t[:], dpatt[:])

            pending = []

            def flush_pending(keep=0):
                # ln for super-block N runs during N+1: its input v is long
                # since ready, so the in-order ACT queue never waits on the
                # DVE chain.  Output DMA via SWDGE keeps ACT queue free.
                while len(pending) > keep:
                    v, oc, ob = pending.pop(0)
                    o = opool.tile([128, oc], BF16, tag="o")
                    nc.scalar.activation(o[:], v[:], Ln, bias=1.0)
                    nc.gpsimd.dma_start(out[:, ob:ob + oc], o[:])

            PREFETCH = 3
            sb_cols = []
            c0 = 0
            for wdt in widths:
                sb_cols.append(c0)
                c0 += 128 * wdt

            def emit_sb(i, col_base, jblk):
                oc, nbank = 20 * jblk, jblk // 12
                tin = xtiles[i]

                # keep the input stream PREFETCH super-blocks ahead
                nxt = i + PREFETCH
                if nxt < nsb:
                    issue_input(nxt, sb_cols[nxt], widths[nxt])

                # psum: 12 j-blocks in the first 480 cols of each bank;
                # bank preload folds +d into the accumulation
                ps = pspool.tile([128, nbank, 512], F32, tag="ps")
                for t in range(nbank):
                    nc.tensor.matmul(ps[:, t, 0:480], ones_t[:], dpat_t[:],
                                     start=True, stop=False)
                for j in range(jblk):
                    nc.tensor.matmul(
                        ps[:, j // 12, 40 * (j % 12):40 * (j % 12) + 40],
                        tin[:, 128 * j:128 * j + 128], w_t[:],
                        start=False, stop=True,
                    )

                # prev super-block's ln + store go first on the ACT queue
                flush_pending(keep=0)

                # tail in <=2-bank chunks; psum cols per j are (xp, g, k).
                # DVE may read PSUM only once per instruction, so the p-half
                # is first copied (cast) to bf16 SBUF, then squared there at
                # 2x mode; ACT squares the x-half straight from PSUM.
                v = mpool.tile([128, oc], BF16, tag="v")
                t0 = 0
                while t0 < nbank:
                    tn = min(2, nbank - t0)
                    cw = tn * 240          # v cols in this chunk
                    pv = ps[:, t0:t0 + tn, 0:480].rearrange(
                        "p t (j c) -> p t j c", c=40)
                    sx = mpool.tile([128, cw], BF16, tag=f"sx{t0}")
                    cp_ = mpool.tile([128, cw], BF16, tag=f"cp{t0}")
                    sp_ = mpool.tile([128, cw], BF16, tag=f"sp{t0}")
                    sxv = sx[:].rearrange("p (t j c) -> p t j c", t=tn, c=20)
                    cpv = cp_[:].rearrange("p (t j c) -> p t j c", t=tn, c=20)
                    nc.scalar.activation(sxv, pv[:, :, :, 0:20], Square)
                    nc.vector.tensor_copy(cpv, pv[:, :, :, 20:40])
                    nc.vector.tensor_mul(sp_[:], cp_[:], cp_[:])
                    nc.vector.tensor_add(
                        v[:, t0 * 240:t0 * 240 + cw], sx[:], sp_[:])
                    t0 += tn
                pending.append((v, oc, (col_base // 128) * 20))

            for i in range(min(PREFETCH, nsb)):
                issue_input(i, sb_cols[i], widths[i])
            for i, wdt in enumerate(widths):
                emit_sb(i, sb_cols[i], wdt)
            flush_pending()
    nc.compile()
    return nc


# ---------------------------------------------------------------- host glue
def _make_in_maps(x_batch, wcat, ones, dpat):
    B = x_batch.shape[0]
    xpad = np.zeros((B_PAD, N), np.float32)
    xpad[:B] = x_batch
    xh = xpad.astype(NPFP8)
    in_maps = []
    for c in range(NCORES):
        xc = xh[c * R:(c + 1) * R]
        xstk = np.empty((128, CC), NPFP8)
        # per sb: rows (grp, t, f) -> xstk[grp*64+f, c0+t]
        r0 = c0 = 0
        for w in WIDTHS:
            half = 128 * w
            xt = xc[r0:r0 + 2 * half].reshape(2, half, N)
            xstk[:, c0:c0 + half] = xt.transpose(0, 2, 1).reshape(128, half)
            r0 += 2 * half
            c0 += half
        in_maps.append({"xs": xstk, "wcat": wcat, "ones": ones,
                        "dpat": dpat})
    return in_maps


def _decode_out(results, B, beta):
    full = np.empty((B_PAD, OUT), np.float32)
    for c in range(NCORES):
        O = results[c]["out"].astype(np.float32)
        r0 = o0 = 0
        for w in WIDTHS:
            Ow = O[:, o0:o0 + 20 * w].reshape(128, w, 2, OUT)
            full[c * R + r0:c * R + r0 + 256 * w] = (
                Ow.transpose(2, 1, 0, 3).reshape(256 * w, OUT))
            r0 += 256 * w
            o0 += 20 * w
    full += beta            # ln1p(covc), factored off the device
    return full[:B]


_NC_CACHE = {}


def kernel(x_batch, int1_0, squeezes_0, int2_0, disp_0,
           int1_1, squeezes_1, int2_1, disp_1, _trace=False):
    layers = [
        (np.asarray(int1_0, np.float64), np.asarray(squeezes_0, np.float64),
         np.asarray(int2_0, np.float64), np.asarray(disp_0, np.float64)),
        (np.asarray(int1_1, np.float64), np.asarray(squeezes_1, np.float64),
         np.asarray(int2_1, np.float64), np.asarray(disp_1, np.float64)),
    ]
    wcat, ones, dpat, beta = _device_constants(layers)
    in_maps = _make_in_maps(np.asarray(x_batch, np.float32),
                            wcat, ones, dpat)

    if "nc" not in _NC_CACHE:
        _NC_CACHE["nc"] = build_nc()
    nc = _NC_CACHE["nc"]

    res = run_bass_kernel_spmd(
        nc, in_maps, core_ids=list(range(NCORES)), trace=_trace
    )
    out = _decode_out(res.results, x_batch.shape[0], beta)
    if _trace:
        return out, res
    return out
